# revision 1
# baseline (speedup 1.0000x reference)
import kernel_lib


def kernel(**inputs):
    return kernel_lib.kernel(inputs['x'], inputs['params'])
